# revision 47
# baseline (speedup 1.0000x reference)
"""Causal self-attention (B=4, S=2048, E=1024, H=16) on 8 trn2 NeuronCores.

Sharding: data parallel over batch (4) x tensor parallel over head groups (2).
Core c handles batch c//2, heads [(c%2)*8, (c%2)*8+8).  Each core computes its
group's QKV projections, causal attention, and a partial output projection;
the host sums the two group partials per batch and adds bo.

Single software-pipelined pass over four 512-token blocks:

    stage qb:  xt-prefetch(qb+1) | QKV(qb) | OPROJ(qb-1) | attention(qb)

so the ScalarE exp stream (the serial bottleneck of a phase-separated
schedule) always overlaps PE matmul work from an adjacent stage.  Engine
budget per core: PE ~250us of matmuls, ACT ~150us of exp, DVE ~110us of
bias/mask/normalize — PE-bound end to end.

Datapath: everything 16-bit is fp16, not bf16 — early tokens have O(1)
attended magnitudes vs a 0.07-std output, so bf16's 0.4% quantization blows
the 2e-2 gate while fp16's 0.05% fits (exp of the max score ~678 is far from
fp16's 65504).  x/Wqkv/Q/K/V/probs/attended/Wo in fp16, all matmuls at full
PE rate with fp32 PSUM accumulation; output partials in fp32.  Measured rel
err ~1.6e-2 vs the 2e-2 gate, HW exec ~306us (from 437us).

Key device tricks kept from / added over the phase-separated version:
  - scores computed transposed [k, q] per 128-k-tile with the head pair at PE
    row groups 0/64 (independent 64-row stationaries, no 128-row LDW).
  - softmax denominator from a 65th ones-column in the AV stationary; the raw
    PSUM row DMAs to DRAM and back as a 64-partition broadcast, then the fast
    custom-DVE reciprocal and one multiply produce the normalized head slice.
    The kernel-tail-critical last pair instead broadcasts the denominator with
    an exact fp32 contract-1 matmul into its own dead a_t bank, skipping the
    DRAM round-trip on the critical path to the final output projection.
  - a single full-tile DVE copy of each AV accumulator frees its PSUM bank in
    ~700ns, keeping the DRAM-bounce latency off the bank-recycle path (this
    alone was worth ~50us).
  - a global AV-lag queue carries pending attention-value matmuls across
    pair/stage boundaries so the PE never drains waiting on the exp stream;
    the previous block's output projection is emitted after the next block's
    first pair for the same reason.
  - causal masking: dead left columns memset on GpSimd BEFORE the exp writes
    the live region (off the exp->AV chain); the 128-wide triangle multiply
    runs on VectorE.
  - Q/K bias folds ride the ScalarE activation (per-partition bias AP +
    scale) that moves them out of PSUM, keeping the DVE queue short for the
    latency-critical triangle/normalize ops.
  - QKV weight/x DMAs issued e-slice-interleaved so the first matmul starts
    ~2us in instead of waiting for the full 8MB input load.
"""

import numpy as np
import ml_dtypes

import concourse.mybir as mybir
from bass_rust import RuntimeValue
import concourse.tile as tile
from concourse import bacc
from concourse.bass_utils import run_bass_kernel_spmd

F32 = mybir.dt.float32
F32R = mybir.dt.float32r
BF16 = mybir.dt.bfloat16
F16 = mybir.dt.float16
Exp = mybir.ActivationFunctionType.Exp
MULT = mybir.AluOpType.mult
ADD = mybir.AluOpType.add

B, S, E, H = 4, 2048, 1024, 16
D = 64          # head dim
HG = 8          # heads per core
G = 512         # group feature width
P = 128
NKT = S // P    # 16 k-tiles
QB = 512        # q/token-block width
NQB = S // QB   # 4
ESUB = E // P   # 8
VW = D + 1      # V stationary width (64 dims + ones column)
AV_LAG = 4      # k-tile slots the AV matmuls trail the score matmuls by

_CACHE = {}


def _build_program():
    nc = bacc.Bacc("TRN2", target_bir_lowering=False, debug=False)

    xt_d = nc.dram_tensor("xt", [E, S], F16, kind="ExternalInput").ap()
    wq_d = nc.dram_tensor("wq", [E, G], F16, kind="ExternalInput").ap()
    wk_d = nc.dram_tensor("wk", [E, G], F16, kind="ExternalInput").ap()
    wv_d = nc.dram_tensor("wv", [E, G], F16, kind="ExternalInput").ap()
    wo_d = nc.dram_tensor("wo", [G, E], F16, kind="ExternalInput").ap()
    bq_d = nc.dram_tensor("bq", [P, 4], F32, kind="ExternalInput").ap()
    bk_d = nc.dram_tensor("bk", [P, 4], F32, kind="ExternalInput").ap()
    bv_d = nc.dram_tensor("bv", [P, G], F32, kind="ExternalInput").ap()
    tri_d = nc.dram_tensor("tri", [P, 2, P], F16, kind="ExternalInput").ap()
    on65_d = nc.dram_tensor("on65", [VW, D], F32, kind="ExternalInput").ap()
    out_d = nc.dram_tensor("out", [S, E], F32, kind="ExternalOutput").ap()
    # scratch for the denominator broadcast bounce
    rc_d = nc.dram_tensor("rc_scratch", [HG, NQB, QB], F32, kind="Internal").ap()

    qt_sb = nc.alloc_sbuf_tensor("qt_sb", [P, 4, S], F16).ap()
    kt_sb = nc.alloc_sbuf_tensor("kt_sb", [P, 4, S], F16).ap()
    vx_sb = nc.alloc_sbuf_tensor("vx_sb", [P, NKT, HG, VW], F16).ap()
    at_sb = nc.alloc_sbuf_tensor("at_sb", [P, 4, S], F16).ap()
    tri_sb = nc.alloc_sbuf_tensor("tri_sb", [P, 2, P], F16).ap()
    on65_sb = nc.alloc_sbuf_tensor("on65_sb", [VW, D], F32).ap()
    bq_sb = nc.alloc_sbuf_tensor("bq_sb", [P, 4], F32).ap()
    bk_sb = nc.alloc_sbuf_tensor("bk_sb", [P, 4], F32).ap()
    bv_sb = nc.alloc_sbuf_tensor("bv_sb", [P, G], F32).ap()

    wq_r = wq_d.rearrange("(o p) f -> p o f", p=P)
    wk_r = wk_d.rearrange("(o p) f -> p o f", p=P)
    wv_r = wv_d.rearrange("(o p) f -> p o f", p=P)
    xt_r = xt_d.rearrange("(o p) s -> p o s", p=P)

    with tile.TileContext(nc) as tc:
        with (
            tc.tile_pool(name="w_pool", bufs=1) as wp,
            tc.tile_pool(name="xt_pool", bufs=2) as xp,
            tc.tile_pool(name="big_ps", bufs=3, space="PSUM") as bp,
            tc.tile_pool(name="a_ps", bufs=2, space="PSUM") as apl,
            tc.tile_pool(name="sp", bufs=2) as sp,
        ):
            wqp = wp.tile([P, ESUB, G], F16, tag="wq", name="wqp")
            wkp = wp.tile([P, ESUB, G], F16, tag="wk", name="wkp")
            wvp = wp.tile([P, ESUB, G], F16, tag="wv", name="wvp")
            wop = wp.tile([P, 4, E], F16, tag="wo", name="wop")
            xtp0 = xp.tile([P, ESUB, QB], F16, tag="xt", name="xtp")
            # wq/x e-slices interleaved so Q(cc=0) can start ~2us in; the
            # rest of the first-chunk input streams underneath the matmuls
            for e in range(ESUB):
                nc.sync.dma_start(wqp[:, e, :], wq_r[:, e, :])
                nc.sync.dma_start(xtp0[:, e, :], xt_r[:, e, 0:QB])
            nc.sync.dma_start(bq_sb[:], bq_d[:])
            nc.sync.dma_start(bk_sb[:], bk_d[:])
            nc.sync.dma_start(bv_sb[:], bv_d[:])
            nc.sync.dma_start(tri_sb[:], tri_d[:])
            nc.sync.dma_start(on65_sb[:], on65_d[:])
            nc.sync.dma_start(wkp[:, 0:4, :], wk_r[:, 0:4, :])
            nc.sync.dma_start(wkp[:, 4:8, :], wk_r[:, 4:8, :])
            nc.sync.dma_start(wvp[:, 0:4, :], wv_r[:, 0:4, :])
            nc.sync.dma_start(wvp[:, 4:8, :], wv_r[:, 4:8, :])
            # ones column of every AV stationary, written once
            nc.vector.memset(
                vx_sb[:, :, :, D : D + 1].rearrange("p k h u -> p (k h) u"), 1.0
            )
            nc.sync.dma_start(wop[:], wo_d.rearrange("(o p) n -> p o n", p=P))

            def qk_part(qb, xtp):
                # e-outer/cc-inner: 4 matmuls per e-slice matches the DMA
                # arrival cadence at startup, so Q never stalls on wq slices
                # and K can start on the first wk half-load
                for w_p, b_sb, dst_sb, scale in (
                    (wqp, bq_sb, qt_sb, 0.125),
                    (wkp, bk_sb, kt_sb, 1.0),
                ):
                    t_a = bp.tile([P, 2, QB], F32, tag="work", name="qk_ps")
                    t_b = bp.tile([P, 2, QB], F32, tag="work", name="qk_ps")
                    for e in range(ESUB):
                        for cc in range(4):
                            nc.tensor.matmul(
                                (t_a if cc < 2 else t_b)[:, cc % 2, :],
                                lhsT=w_p[:, e, cc * P : (cc + 1) * P],
                                rhs=xtp[:, e, :],
                                start=(e == 0),
                                stop=(e == ESUB - 1),
                            )
                    for cc in range(4):
                        nc.scalar.activation(
                            dst_sb[:, cc, qb * QB : (qb + 1) * QB],
                            (t_a if cc < 2 else t_b)[:, cc % 2, :],
                            mybir.ActivationFunctionType.Identity,
                            bias=b_sb[:, cc : cc + 1],
                            scale=scale,
                        )

            def v_part(qb, xtp):
                for half in range(2):
                    v2 = bp.tile([P, 2, QB], F32, tag="work", name="v_ps")
                    for sub in range(2):
                        stl = half * 2 + sub
                        for e in range(ESUB):
                            nc.tensor.matmul(
                                v2[:, sub, :],
                                lhsT=xtp[:, e, stl * P : (stl + 1) * P],
                                rhs=wvp[:, e, :],
                                start=(e == 0),
                                stop=(e == ESUB - 1),
                            )
                    for sub in range(2):
                        st = qb * 4 + half * 2 + sub
                        nc.vector.tensor_tensor(
                            vx_sb[:, st, :, 0:D],
                            v2[:, sub, :].rearrange("p (h d) -> p h d", d=D),
                            bv_sb.rearrange("p (h d) -> p h d", d=D),
                            ADD,
                        )

            def attn_stage(qb, pairs=(0, 1, 2, 3), post_pair0=None):
                for pair in pairs:
                    if pair == 1 and post_pair0 is not None:
                        post_pair0()
                    heads = (2 * pair, 2 * pair + 1)
                    a_t = {
                        h: apl.tile([VW, QB], F32, tag="a", name="a_ps") for h in heads
                    }
                    nkt = 4 * qb + 4

                    def mk_av(a_t, heads, pair, qb, nkt):
                        def av_mms(pt_prev, kt_prev):
                            for h in heads:
                                nc.tensor.matmul(
                                    a_t[h][:],
                                    lhsT=vx_sb[:, kt_prev, h, :],
                                    rhs=pt_prev[:, h % 2, :],
                                    start=(kt_prev == 0),
                                    stop=(kt_prev == nkt - 1),
                                )
                            if kt_prev == nkt - 1:
                                normalize(a_t, heads, pair, qb)
                        return av_mms

                    av_mms = mk_av(a_t, heads, pair, qb, nkt)
                    for kt in range(nkt):
                        s_t = bp.tile([P, 2, QB], F32, tag="work", name="s_ps")
                        for h in heads:
                            hb = (h % 2) * D
                            nc.tensor.matmul(
                                s_t[:, h % 2, :],
                                lhsT=kt_sb[hb : hb + D, pair, kt * P : (kt + 1) * P],
                                rhs=qt_sb[hb : hb + D, pair, qb * QB : (qb + 1) * QB],
                                start=True,
                                stop=True,
                            )
                        pt = sp.tile([P, 2, QB], F16, tag="pt", name="pt", bufs=8)
                        m = kt - 4 * qb  # >= 0 only on diagonal k-tiles
                        if m > 0:
                            # dead left cols: zero on gpsimd BEFORE the exp
                            # writes the live region — off the exp->AV chain
                            nc.gpsimd.memset(pt[:, :, 0 : P * m], 0.0)
                            nc.scalar.activation(
                                pt[:, :, P * m :], s_t[:, :, P * m :], Exp
                            )
                        else:
                            nc.scalar.activation(pt[:], s_t[:], Exp)
                        if m >= 0:
                            # multiply the 128-wide triangle into both heads
                            # with a single DVE op
                            nc.vector.tensor_tensor(
                                pt[:, :, P * m : P * (m + 1)],
                                pt[:, :, P * m : P * (m + 1)],
                                tri_sb[:],
                                MULT,
                            )
                        if len(pending) >= AV_LAG:
                            pending.pop(0)()
                        pending.append(lambda pt=pt, kt=kt, f=av_mms: f(pt, kt))

            def normalize(a_t, heads, pair, qb):
                # one full-tile copy frees the PSUM bank in ~700ns; the
                # denominator DRAM-broadcast + reciprocal + scale run off
                # the SBUF copy, clear of the a_t bank recycle path
                fast = pair == 3 and qb == NQB - 1
                an = {}
                for h in heads:
                    an[h] = sp.tile([VW, QB], F32, tag="dn", name="an", bufs=4)
                    nc.vector.tensor_copy(an[h][:], a_t[h][:])
                    if not fast:
                        nc.sync.dma_start(rc_d[h, qb : qb + 1, :], an[h][D:VW, :])
                for h in heads:
                    hb = (h % 2) * D
                    rb = sp.tile([D, QB], F32, tag="rb", name="rb", bufs=2)
                    if fast:
                        # exact fp32 contract-1 matmul broadcasts the
                        # denominator row into the dead a_t bank: the kernel
                        # tail skips the DRAM round-trip entirely
                        nc.tensor.matmul(
                            a_t[h][0:D, :],
                            lhsT=on65_sb[D:VW, :],
                            rhs=an[h][D:VW, :],
                            start=True,
                            stop=True,
                        )
                        nc.vector.reciprocal_approx_fast(rb[:], a_t[h][0:D, :])
                    else:
                        db = sp.tile([D, QB], F32, tag="db", name="db", bufs=2)
                        nc.sync.dma_start(
                            db[:], rc_d[h, qb : qb + 1, :].to_broadcast([D, QB])
                        )
                        nc.vector.reciprocal_approx_fast(rb[:], db[:])
                    at_slice = at_sb[hb : hb + D, pair, qb * QB : (qb + 1) * QB]
                    if hb == 0:
                        nc.vector.tensor_tensor(at_slice, an[h][0:D, :], rb[:], MULT)
                    else:
                        tmp = sp.tile([D, QB], F16, tag="tmp", name="tmp", bufs=2)
                        nc.vector.tensor_tensor(tmp[:], an[h][0:D, :], rb[:], MULT)
                        nc.sync.dma_start(at_slice, tmp[:])

            def oproj_stage(qb):
                for stl in range(4):
                    st = qb * 4 + stl
                    o = bp.tile([P, 2, QB], F32, tag="work", name="o_ps")
                    for n in range(2):
                        for t in range(4):
                            nc.tensor.matmul(
                                o[:, n, :],
                                lhsT=at_sb[:, t, st * P : (st + 1) * P],
                                rhs=wop[:, t, n * QB : (n + 1) * QB],
                                start=(t == 0),
                                stop=(t == 3),
                            )
                    o_sb = sp.tile([P, 2, QB], F32, tag="ob", name="o_sb", bufs=2)
                    for n in range(2):
                        if (st + n) % 2 == 0:
                            nc.scalar.copy(o_sb[:, n, :], o[:, n, :])
                        else:
                            nc.vector.tensor_copy(o_sb[:, n, :], o[:, n, :])
                        nc.sync.dma_start(
                            out_d[st * P : (st + 1) * P, n * QB : (n + 1) * QB],
                            o_sb[:, n, :],
                        )

            pending = []
            for qb in range(NQB):
                if qb + 1 < NQB:
                    nxt = xp.tile([P, ESUB, QB], F16, tag="xt", name="xtp")
                    nc.sync.dma_start(
                        nxt[:], xt_r[:, :, (qb + 1) * QB : (qb + 2) * QB]
                    )
                else:
                    nxt = None
                xtp = xtp0 if qb == 0 else cur
                qk_part(qb, xtp)
                v_part(qb, xtp)
                attn_stage(
                    qb,
                    post_pair0=(
                        (lambda q=qb: oproj_stage(q - 1)) if qb >= 1 else None
                    ),
                )
                cur = nxt
                if qb == NQB - 1:
                    for f in pending:
                        f()
                    pending = []
            oproj_stage(NQB - 1)

    nc.compile()
    return nc


def _prep_inputs(x, Wqkv, bqkv, Wo, bo):
    bf16 = np.float16
    x = np.asarray(x, np.float32)
    Wqkv = np.asarray(Wqkv, np.float32)
    bqkv = np.asarray(bqkv, np.float32)
    Wo = np.asarray(Wo, np.float32)

    kl = np.arange(P)[:, None]
    tri1 = (kl <= np.arange(P)[None, :]).astype(np.float16)
    tri = np.stack([tri1, tri1], axis=1)
    in_maps = []
    for c in range(8):
        b, g = divmod(c, 2)
        lo, hi = g * G, (g + 1) * G
        in_maps.append(
            {
                "xt": np.ascontiguousarray(x[b].T).astype(bf16),
                "wq": np.ascontiguousarray(Wqkv[:, lo:hi]).astype(bf16),
                "wk": np.ascontiguousarray(Wqkv[:, E + lo : E + hi]).astype(bf16),
                "wv": np.ascontiguousarray(Wqkv[:, 2 * E + lo : 2 * E + hi]).astype(
                    bf16
                ),
                "wo": np.ascontiguousarray(Wo[lo:hi, :]).astype(bf16),
                "bq": np.ascontiguousarray(bqkv[lo:hi].reshape(4, P).T) * 0.125,
                "bk": np.ascontiguousarray(bqkv[E + lo : E + hi].reshape(4, P).T),
                "bv": np.tile(bqkv[2 * E + lo : 2 * E + hi][None, :], (P, 1)).astype(
                    np.float32
                ),
                "tri": tri,
                "on65": np.ones((VW, D), np.float32),
            }
        )
    return in_maps


def kernel(x, Wqkv, bqkv, Wo, bo, _trace=False):
    if "nc" not in _CACHE:
        _CACHE["nc"] = _build_program()
    nc = _CACHE["nc"]

    in_maps = _prep_inputs(x, Wqkv, bqkv, Wo, bo)
    res = run_bass_kernel_spmd(nc, in_maps, core_ids=list(range(8)), trace=_trace)
    _CACHE["last_result"] = res

    bo = np.asarray(bo, np.float32)
    out = np.empty((B, S, E), np.float32)
    for b in range(B):
        out[b] = (
            res.results[2 * b]["out"].astype(np.float32)
            + res.results[2 * b + 1]["out"].astype(np.float32)
            + bo
        )
    return out


# revision 48
# speedup vs baseline: 1.0129x; 1.0129x over previous
"""Causal self-attention (B=4, S=2048, E=1024, H=16) on 8 trn2 NeuronCores.

Sharding: data parallel over batch (4) x tensor parallel over head groups (2).
Core c handles batch c//2, heads [(c%2)*8, (c%2)*8+8).  Each core computes its
group's QKV projections, causal attention, and a partial output projection;
the host sums the two group partials per batch and adds bo.

Single software-pipelined pass over four 512-token blocks:

    stage qb:  xt-prefetch(qb+1) | QKV(qb) | OPROJ(qb-1) | attention(qb)

so the ScalarE exp stream (the serial bottleneck of a phase-separated
schedule) always overlaps PE matmul work from an adjacent stage.  Engine
budget per core: PE ~250us of matmuls, ACT ~150us of exp, DVE ~110us of
bias/mask/normalize — PE-bound end to end.

Datapath: everything 16-bit is fp16, not bf16 — early tokens have O(1)
attended magnitudes vs a 0.07-std output, so bf16's 0.4% quantization blows
the 2e-2 gate while fp16's 0.05% fits (exp of the max score ~678 is far from
fp16's 65504).  x/Wqkv/Q/K/V/probs/attended/Wo in fp16, all matmuls at full
PE rate with fp32 PSUM accumulation; output partials in fp32.  Measured rel
err ~1.6e-2 vs the 2e-2 gate, HW exec ~306us (from 437us).

Key device tricks kept from / added over the phase-separated version:
  - scores computed transposed [k, q] per 128-k-tile with the head pair at PE
    row groups 0/64 (independent 64-row stationaries, no 128-row LDW).
  - softmax denominator from a 65th ones-column in the AV stationary; the raw
    PSUM row DMAs to DRAM and back as a 64-partition broadcast, then the fast
    custom-DVE reciprocal and one multiply produce the normalized head slice.
    The kernel-tail-critical last pair instead broadcasts the denominator with
    an exact fp32 contract-1 matmul into its own dead a_t bank, skipping the
    DRAM round-trip on the critical path to the final output projection.
  - a single full-tile DVE copy of each AV accumulator frees its PSUM bank in
    ~700ns, keeping the DRAM-bounce latency off the bank-recycle path (this
    alone was worth ~50us).
  - a global AV-lag queue carries pending attention-value matmuls across
    pair/stage boundaries so the PE never drains waiting on the exp stream;
    the previous block's output projection is emitted after the next block's
    first pair for the same reason.
  - causal masking: dead left columns memset on GpSimd BEFORE the exp writes
    the live region (off the exp->AV chain); the 128-wide triangle multiply
    runs on VectorE.
  - Q/K bias folds ride the ScalarE activation (per-partition bias AP +
    scale) that moves them out of PSUM, keeping the DVE queue short for the
    latency-critical triangle/normalize ops.
  - QKV weight/x DMAs issued e-slice-interleaved so the first matmul starts
    ~2us in instead of waiting for the full 8MB input load.
"""

import numpy as np
import ml_dtypes

import concourse.mybir as mybir
from bass_rust import RuntimeValue
import concourse.tile as tile
from concourse import bacc
from concourse.bass_utils import run_bass_kernel_spmd

F32 = mybir.dt.float32
F32R = mybir.dt.float32r
BF16 = mybir.dt.bfloat16
F16 = mybir.dt.float16
Exp = mybir.ActivationFunctionType.Exp
MULT = mybir.AluOpType.mult
ADD = mybir.AluOpType.add

B, S, E, H = 4, 2048, 1024, 16
D = 64          # head dim
HG = 8          # heads per core
G = 512         # group feature width
P = 128
NKT = S // P    # 16 k-tiles
QB = 512        # q/token-block width
NQB = S // QB   # 4
ESUB = E // P   # 8
VW = D + 1      # V stationary width (64 dims + ones column)
AV_LAG = 4      # k-tile slots the AV matmuls trail the score matmuls by

_CACHE = {}


def _build_program():
    nc = bacc.Bacc("TRN2", target_bir_lowering=False, debug=False)

    xt_d = nc.dram_tensor("xt", [E, S], F16, kind="ExternalInput").ap()
    wq_d = nc.dram_tensor("wq", [E, G], F16, kind="ExternalInput").ap()
    wk_d = nc.dram_tensor("wk", [E, G], F16, kind="ExternalInput").ap()
    wv_d = nc.dram_tensor("wv", [E, G], F16, kind="ExternalInput").ap()
    wo_d = nc.dram_tensor("wo", [G, E], F16, kind="ExternalInput").ap()
    bq_d = nc.dram_tensor("bq", [P, 4], F32, kind="ExternalInput").ap()
    bk_d = nc.dram_tensor("bk", [P, 4], F32, kind="ExternalInput").ap()
    bv_d = nc.dram_tensor("bv", [P, G], F32, kind="ExternalInput").ap()
    tri_d = nc.dram_tensor("tri", [P, 2, P], F16, kind="ExternalInput").ap()
    on65_d = nc.dram_tensor("on65", [VW, D], F32, kind="ExternalInput").ap()
    out_d = nc.dram_tensor("out", [S, E], F32, kind="ExternalOutput").ap()
    # scratch for the denominator broadcast bounce
    rc_d = nc.dram_tensor("rc_scratch", [HG, NQB, QB], F32, kind="Internal").ap()

    qt_sb = nc.alloc_sbuf_tensor("qt_sb", [P, 4, S], F16).ap()
    kt_sb = nc.alloc_sbuf_tensor("kt_sb", [P, 4, S], F16).ap()
    vx_sb = nc.alloc_sbuf_tensor("vx_sb", [P, NKT, HG, VW], F16).ap()
    at_sb = nc.alloc_sbuf_tensor("at_sb", [P, 4, S], F16).ap()
    tri_sb = nc.alloc_sbuf_tensor("tri_sb", [P, 2, P], F16).ap()
    on65_sb = nc.alloc_sbuf_tensor("on65_sb", [VW, D], F32).ap()
    bq_sb = nc.alloc_sbuf_tensor("bq_sb", [P, 4], F32).ap()
    bk_sb = nc.alloc_sbuf_tensor("bk_sb", [P, 4], F32).ap()
    bv_sb = nc.alloc_sbuf_tensor("bv_sb", [P, G], F32).ap()

    wq_r = wq_d.rearrange("(o p) f -> p o f", p=P)
    wk_r = wk_d.rearrange("(o p) f -> p o f", p=P)
    wv_r = wv_d.rearrange("(o p) f -> p o f", p=P)
    xt_r = xt_d.rearrange("(o p) s -> p o s", p=P)

    with tile.TileContext(nc) as tc:
        with (
            tc.tile_pool(name="w_pool", bufs=1) as wp,
            tc.tile_pool(name="xt_pool", bufs=2) as xp,
            tc.tile_pool(name="big_ps", bufs=3, space="PSUM") as bp,
            tc.tile_pool(name="a_ps", bufs=2, space="PSUM") as apl,
            tc.tile_pool(name="sp", bufs=2) as sp,
        ):
            wqp = wp.tile([P, ESUB, G], F16, tag="wq", name="wqp")
            wkp = wp.tile([P, ESUB, G], F16, tag="wk", name="wkp")
            wvp = wp.tile([P, ESUB, G], F16, tag="wv", name="wvp")
            wop = wp.tile([P, 4, E], F16, tag="wo", name="wop")
            xtp0 = xp.tile([P, ESUB, QB], F16, tag="xt", name="xtp")
            # wq/x e-slices interleaved so Q(cc=0) can start ~2us in; the
            # rest of the first-chunk input streams underneath the matmuls
            for e in range(ESUB):
                nc.sync.dma_start(wqp[:, e, :], wq_r[:, e, :])
                nc.sync.dma_start(xtp0[:, e, :], xt_r[:, e, 0:QB])
            nc.sync.dma_start(bq_sb[:], bq_d[:])
            nc.sync.dma_start(bk_sb[:], bk_d[:])
            nc.sync.dma_start(bv_sb[:], bv_d[:])
            nc.sync.dma_start(tri_sb[:], tri_d[:])
            nc.sync.dma_start(on65_sb[:], on65_d[:])
            nc.sync.dma_start(wkp[:, 0:4, :], wk_r[:, 0:4, :])
            nc.sync.dma_start(wkp[:, 4:8, :], wk_r[:, 4:8, :])
            nc.sync.dma_start(wvp[:, 0:4, :], wv_r[:, 0:4, :])
            nc.sync.dma_start(wvp[:, 4:8, :], wv_r[:, 4:8, :])
            # ones column of every AV stationary, written once
            nc.vector.memset(
                vx_sb[:, :, :, D : D + 1].rearrange("p k h u -> p (k h) u"), 1.0
            )
            nc.sync.dma_start(wop[:], wo_d.rearrange("(o p) n -> p o n", p=P))

            def qk_part(qb, xtp):
                # e-outer/cc-inner: 4 matmuls per e-slice matches the DMA
                # arrival cadence at startup, so Q never stalls on wq slices
                # and K can start on the first wk half-load
                for w_p, b_sb, dst_sb, scale in (
                    (wqp, bq_sb, qt_sb, 0.125),
                    (wkp, bk_sb, kt_sb, 1.0),
                ):
                    t_a = bp.tile([P, 2, QB], F32, tag="work", name="qk_ps")
                    t_b = bp.tile([P, 2, QB], F32, tag="work", name="qk_ps")
                    for e in range(ESUB):
                        for cc in range(4):
                            nc.tensor.matmul(
                                (t_a if cc < 2 else t_b)[:, cc % 2, :],
                                lhsT=w_p[:, e, cc * P : (cc + 1) * P],
                                rhs=xtp[:, e, :],
                                start=(e == 0),
                                stop=(e == ESUB - 1),
                            )
                    for cc in range(4):
                        nc.scalar.activation(
                            dst_sb[:, cc, qb * QB : (qb + 1) * QB],
                            (t_a if cc < 2 else t_b)[:, cc % 2, :],
                            mybir.ActivationFunctionType.Identity,
                            bias=b_sb[:, cc : cc + 1],
                            scale=scale,
                        )

            def v_part(qb, xtp):
                for half in range(2):
                    v2 = bp.tile([P, 2, QB], F32, tag="work", name="v_ps")
                    for sub in range(2):
                        stl = half * 2 + sub
                        for e in range(ESUB):
                            nc.tensor.matmul(
                                v2[:, sub, :],
                                lhsT=xtp[:, e, stl * P : (stl + 1) * P],
                                rhs=wvp[:, e, :],
                                start=(e == 0),
                                stop=(e == ESUB - 1),
                            )
                    for sub in range(2):
                        st = qb * 4 + half * 2 + sub
                        nc.vector.tensor_tensor(
                            vx_sb[:, st, :, 0:D],
                            v2[:, sub, :].rearrange("p (h d) -> p h d", d=D),
                            bv_sb.rearrange("p (h d) -> p h d", d=D),
                            ADD,
                        )

            def attn_stage(qb, pairs=(0, 1, 2, 3), post_pair0=None):
                for pair in pairs:
                    if pair == 1 and post_pair0 is not None:
                        post_pair0()
                    heads = (2 * pair, 2 * pair + 1)
                    a_t = {
                        h: apl.tile([VW, QB], F32, tag="a", name="a_ps") for h in heads
                    }
                    nkt = 4 * qb + 4

                    def mk_av(a_t, heads, pair, qb, nkt):
                        def av_mms(pt_prev, kt_prev):
                            for h in heads:
                                nc.tensor.matmul(
                                    a_t[h][:],
                                    lhsT=vx_sb[:, kt_prev, h, :],
                                    rhs=pt_prev[:, h % 2, :],
                                    start=(kt_prev == 0),
                                    stop=(kt_prev == nkt - 1),
                                )
                            if kt_prev == nkt - 1:
                                normalize(a_t, heads, pair, qb)
                        return av_mms

                    av_mms = mk_av(a_t, heads, pair, qb, nkt)
                    for kt in range(nkt):
                        s_t = bp.tile([P, 2, QB], F32, tag="work", name="s_ps")
                        for h in heads:
                            hb = (h % 2) * D
                            nc.tensor.matmul(
                                s_t[:, h % 2, :],
                                lhsT=kt_sb[hb : hb + D, pair, kt * P : (kt + 1) * P],
                                rhs=qt_sb[hb : hb + D, pair, qb * QB : (qb + 1) * QB],
                                start=True,
                                stop=True,
                            )
                        pt = sp.tile([P, 2, QB], F16, tag="pt", name="pt", bufs=8)
                        m = kt - 4 * qb  # >= 0 only on diagonal k-tiles
                        if m > 0:
                            # dead left cols: zero on gpsimd BEFORE the exp
                            # writes the live region — off the exp->AV chain
                            nc.gpsimd.memset(pt[:, :, 0 : P * m], 0.0)
                            nc.scalar.activation(
                                pt[:, :, P * m :], s_t[:, :, P * m :], Exp
                            )
                        else:
                            nc.scalar.activation(pt[:], s_t[:], Exp)
                        if m >= 0:
                            # multiply the 128-wide triangle into both heads
                            # with a single DVE op
                            nc.vector.tensor_tensor(
                                pt[:, :, P * m : P * (m + 1)],
                                pt[:, :, P * m : P * (m + 1)],
                                tri_sb[:],
                                MULT,
                            )
                        if len(pending) >= AV_LAG:
                            pending.pop(0)()
                        pending.append(lambda pt=pt, kt=kt, f=av_mms: f(pt, kt))

            def normalize(a_t, heads, pair, qb):
                # one full-tile copy frees the PSUM bank in ~700ns; the
                # denominator DRAM-broadcast + reciprocal + scale run off
                # the SBUF copy, clear of the a_t bank recycle path
                fast = pair == 3 and qb == NQB - 1
                # odd head first: its chain ends in an at_sb DMA that gates
                # the output projection, so give it the head start
                order = (heads[1], heads[0])
                an = {}
                for h in order:
                    an[h] = sp.tile([VW, QB], F32, tag="dn", name="an", bufs=4)
                    nc.vector.tensor_copy(an[h][:], a_t[h][:])
                    if not fast:
                        nc.sync.dma_start(rc_d[h, qb : qb + 1, :], an[h][D:VW, :])
                for h in order:
                    hb = (h % 2) * D
                    rb = sp.tile([D, QB], F32, tag="rb", name="rb", bufs=2)
                    if fast:
                        # exact fp32 contract-1 matmul broadcasts the
                        # denominator row into the dead a_t bank: the kernel
                        # tail skips the DRAM round-trip entirely
                        nc.tensor.matmul(
                            a_t[h][0:D, :],
                            lhsT=on65_sb[D:VW, :],
                            rhs=an[h][D:VW, :],
                            start=True,
                            stop=True,
                        )
                        nc.vector.reciprocal_approx_fast(rb[:], a_t[h][0:D, :])
                    else:
                        db = sp.tile([D, QB], F32, tag="db", name="db", bufs=2)
                        nc.sync.dma_start(
                            db[:], rc_d[h, qb : qb + 1, :].to_broadcast([D, QB])
                        )
                        nc.vector.reciprocal_approx_fast(rb[:], db[:])
                    at_slice = at_sb[hb : hb + D, pair, qb * QB : (qb + 1) * QB]
                    if hb == 0:
                        nc.vector.tensor_tensor(at_slice, an[h][0:D, :], rb[:], MULT)
                    else:
                        tmp = sp.tile([D, QB], F16, tag="tmp", name="tmp", bufs=2)
                        nc.vector.tensor_tensor(tmp[:], an[h][0:D, :], rb[:], MULT)
                        nc.sync.dma_start(at_slice, tmp[:])

            def oproj_stage(qb):
                for stl in range(4):
                    st = qb * 4 + stl
                    o = bp.tile([P, 2, QB], F32, tag="work", name="o_ps")
                    for n in range(2):
                        for t in range(4):
                            nc.tensor.matmul(
                                o[:, n, :],
                                lhsT=at_sb[:, t, st * P : (st + 1) * P],
                                rhs=wop[:, t, n * QB : (n + 1) * QB],
                                start=(t == 0),
                                stop=(t == 3),
                            )
                    o_sb = sp.tile([P, 2, QB], F32, tag="ob", name="o_sb", bufs=2)
                    for n in range(2):
                        if (st + n) % 2 == 0:
                            nc.scalar.copy(o_sb[:, n, :], o[:, n, :])
                        else:
                            nc.vector.tensor_copy(o_sb[:, n, :], o[:, n, :])
                        nc.sync.dma_start(
                            out_d[st * P : (st + 1) * P, n * QB : (n + 1) * QB],
                            o_sb[:, n, :],
                        )

            pending = []
            for qb in range(NQB):
                if qb + 1 < NQB:
                    nxt = xp.tile([P, ESUB, QB], F16, tag="xt", name="xtp")
                    nc.sync.dma_start(
                        nxt[:], xt_r[:, :, (qb + 1) * QB : (qb + 2) * QB]
                    )
                else:
                    nxt = None
                xtp = xtp0 if qb == 0 else cur
                qk_part(qb, xtp)
                v_part(qb, xtp)
                attn_stage(
                    qb,
                    post_pair0=(
                        (lambda q=qb: oproj_stage(q - 1)) if qb >= 1 else None
                    ),
                )
                cur = nxt
                if qb == NQB - 1:
                    for f in pending:
                        f()
                    pending = []
            oproj_stage(NQB - 1)

    nc.compile()
    return nc


def _prep_inputs(x, Wqkv, bqkv, Wo, bo):
    bf16 = np.float16
    x = np.asarray(x, np.float32)
    Wqkv = np.asarray(Wqkv, np.float32)
    bqkv = np.asarray(bqkv, np.float32)
    Wo = np.asarray(Wo, np.float32)

    kl = np.arange(P)[:, None]
    tri1 = (kl <= np.arange(P)[None, :]).astype(np.float16)
    tri = np.stack([tri1, tri1], axis=1)
    in_maps = []
    for c in range(8):
        b, g = divmod(c, 2)
        lo, hi = g * G, (g + 1) * G
        in_maps.append(
            {
                "xt": np.ascontiguousarray(x[b].T).astype(bf16),
                "wq": np.ascontiguousarray(Wqkv[:, lo:hi]).astype(bf16),
                "wk": np.ascontiguousarray(Wqkv[:, E + lo : E + hi]).astype(bf16),
                "wv": np.ascontiguousarray(Wqkv[:, 2 * E + lo : 2 * E + hi]).astype(
                    bf16
                ),
                "wo": np.ascontiguousarray(Wo[lo:hi, :]).astype(bf16),
                "bq": np.ascontiguousarray(bqkv[lo:hi].reshape(4, P).T) * 0.125,
                "bk": np.ascontiguousarray(bqkv[E + lo : E + hi].reshape(4, P).T),
                "bv": np.tile(bqkv[2 * E + lo : 2 * E + hi][None, :], (P, 1)).astype(
                    np.float32
                ),
                "tri": tri,
                "on65": np.ones((VW, D), np.float32),
            }
        )
    return in_maps


def kernel(x, Wqkv, bqkv, Wo, bo, _trace=False):
    if "nc" not in _CACHE:
        _CACHE["nc"] = _build_program()
    nc = _CACHE["nc"]

    in_maps = _prep_inputs(x, Wqkv, bqkv, Wo, bo)
    res = run_bass_kernel_spmd(nc, in_maps, core_ids=list(range(8)), trace=_trace)
    _CACHE["last_result"] = res

    bo = np.asarray(bo, np.float32)
    out = np.empty((B, S, E), np.float32)
    for b in range(B):
        out[b] = (
            res.results[2 * b]["out"].astype(np.float32)
            + res.results[2 * b + 1]["out"].astype(np.float32)
            + bo
        )
    return out


# revision 50
# speedup vs baseline: 1.0304x; 1.0174x over previous
"""Causal self-attention (B=4, S=2048, E=1024, H=16) on 8 trn2 NeuronCores.

Sharding: data parallel over batch (4) x tensor parallel over head groups (2).
Core c handles batch c//2, heads [(c%2)*8, (c%2)*8+8).  Each core computes its
group's QKV projections, causal attention, and a partial output projection;
the host sums the two group partials per batch and adds bo.

Single software-pipelined pass over four 512-token blocks:

    stage qb:  xt-prefetch(qb+1) | QKV(qb) | OPROJ(qb-1) | attention(qb)

so the ScalarE exp stream (the serial bottleneck of a phase-separated
schedule) always overlaps PE matmul work from an adjacent stage.  Engine
budget per core: PE ~250us of matmuls, ACT ~150us of exp, DVE ~110us of
bias/mask/normalize — PE-bound end to end.

Datapath: everything 16-bit is fp16, not bf16 — early tokens have O(1)
attended magnitudes vs a 0.07-std output, so bf16's 0.4% quantization blows
the 2e-2 gate while fp16's 0.05% fits (exp of the max score ~678 is far from
fp16's 65504).  x/Wqkv/Q/K/V/probs/attended/Wo in fp16, all matmuls at full
PE rate with fp32 PSUM accumulation; output partials in fp32.  Measured rel
err ~1.6e-2 vs the 2e-2 gate, HW exec ~303us (from 437us).

Key device tricks kept from / added over the phase-separated version:
  - scores computed transposed [k, q] per 128-k-tile with the head pair at PE
    row groups 0/64 (independent 64-row stationaries, no 128-row LDW).
  - softmax denominator from a 65th ones-column in the AV stationary; the raw
    PSUM row DMAs to DRAM and back as a 64-partition broadcast, then the fast
    custom-DVE reciprocal and one multiply produce the normalized head slice.
    The kernel-tail-critical last pair instead broadcasts the denominator with
    an exact fp32 contract-1 matmul into its own dead a_t bank, skipping the
    DRAM round-trip on the critical path to the final output projection.
  - a single full-tile DVE copy of each AV accumulator frees its PSUM bank in
    ~700ns, keeping the DRAM-bounce latency off the bank-recycle path (this
    alone was worth ~50us).
  - a global AV-lag queue carries pending attention-value matmuls across
    pair/stage boundaries so the PE never drains waiting on the exp stream;
    the previous block's output projection is emitted after the next block's
    first pair for the same reason.
  - causal masking: dead left columns memset on GpSimd BEFORE the exp writes
    the live region (off the exp->AV chain); the 128-wide triangle multiply
    runs on VectorE.
  - Q/K bias folds ride the ScalarE activation (per-partition bias AP +
    scale) that moves them out of PSUM, keeping the DVE queue short for the
    latency-critical triangle/normalize ops.
  - QKV weight/x DMAs issued e-slice-interleaved so the first matmul starts
    ~2us in instead of waiting for the full 8MB input load.
"""

import numpy as np
import ml_dtypes

import concourse.mybir as mybir
from bass_rust import RuntimeValue
import concourse.tile as tile
from concourse import bacc
from concourse.bass_utils import run_bass_kernel_spmd

F32 = mybir.dt.float32
F32R = mybir.dt.float32r
BF16 = mybir.dt.bfloat16
F16 = mybir.dt.float16
Exp = mybir.ActivationFunctionType.Exp
MULT = mybir.AluOpType.mult
ADD = mybir.AluOpType.add

B, S, E, H = 4, 2048, 1024, 16
D = 64          # head dim
HG = 8          # heads per core
G = 512         # group feature width
P = 128
NKT = S // P    # 16 k-tiles
QB = 512        # q/token-block width
NQB = S // QB   # 4
ESUB = E // P   # 8
VW = D + 1      # V stationary width (64 dims + ones column)
AV_LAG = 4      # k-tile slots the AV matmuls trail the score matmuls by

_CACHE = {}


def _build_program():
    nc = bacc.Bacc("TRN2", target_bir_lowering=False, debug=False)

    xt_d = nc.dram_tensor("xt", [E, S], F16, kind="ExternalInput").ap()
    wq_d = nc.dram_tensor("wq", [E, G], F16, kind="ExternalInput").ap()
    wk_d = nc.dram_tensor("wk", [E, G], F16, kind="ExternalInput").ap()
    wv_d = nc.dram_tensor("wv", [E, G], F16, kind="ExternalInput").ap()
    wo_d = nc.dram_tensor("wo", [G, E], F16, kind="ExternalInput").ap()
    bq_d = nc.dram_tensor("bq", [P, 4], F32, kind="ExternalInput").ap()
    bk_d = nc.dram_tensor("bk", [P, 4], F32, kind="ExternalInput").ap()
    bv_d = nc.dram_tensor("bv", [P, G], F32, kind="ExternalInput").ap()
    tri_d = nc.dram_tensor("tri", [P, 2, P], F16, kind="ExternalInput").ap()
    on65_d = nc.dram_tensor("on65", [VW, D], F32, kind="ExternalInput").ap()
    out_d = nc.dram_tensor("out", [S, E], F32, kind="ExternalOutput").ap()
    # last token block rides its own f16 output: halves the final DMA drain;
    # late tokens are small-magnitude so the f16 step is ~1e-4 there
    out2_d = nc.dram_tensor("out2", [QB, E], F16, kind="ExternalOutput").ap()
    # scratch for the denominator broadcast bounce
    rc_d = nc.dram_tensor("rc_scratch", [HG, NQB, QB], F32, kind="Internal").ap()

    qt_sb = nc.alloc_sbuf_tensor("qt_sb", [P, 4, S], F16).ap()
    kt_sb = nc.alloc_sbuf_tensor("kt_sb", [P, 4, S], F16).ap()
    vx_sb = nc.alloc_sbuf_tensor("vx_sb", [P, NKT, HG, VW], F16).ap()
    at_sb = nc.alloc_sbuf_tensor("at_sb", [P, 4, S], F16).ap()
    tri_sb = nc.alloc_sbuf_tensor("tri_sb", [P, 2, P], F16).ap()
    on65_sb = nc.alloc_sbuf_tensor("on65_sb", [VW, D], F32).ap()
    bq_sb = nc.alloc_sbuf_tensor("bq_sb", [P, 4], F32).ap()
    bk_sb = nc.alloc_sbuf_tensor("bk_sb", [P, 4], F32).ap()
    bv_sb = nc.alloc_sbuf_tensor("bv_sb", [P, G], F32).ap()

    wq_r = wq_d.rearrange("(o p) f -> p o f", p=P)
    wk_r = wk_d.rearrange("(o p) f -> p o f", p=P)
    wv_r = wv_d.rearrange("(o p) f -> p o f", p=P)
    xt_r = xt_d.rearrange("(o p) s -> p o s", p=P)

    with tile.TileContext(nc) as tc:
        with (
            tc.tile_pool(name="w_pool", bufs=1) as wp,
            tc.tile_pool(name="xt_pool", bufs=2) as xp,
            tc.tile_pool(name="big_ps", bufs=3, space="PSUM") as bp,
            tc.tile_pool(name="a_ps", bufs=2, space="PSUM") as apl,
            tc.tile_pool(name="sp", bufs=2) as sp,
        ):
            wqp = wp.tile([P, ESUB, G], F16, tag="wq", name="wqp")
            wkp = wp.tile([P, ESUB, G], F16, tag="wk", name="wkp")
            wvp = wp.tile([P, ESUB, G], F16, tag="wv", name="wvp")
            wop = wp.tile([P, 4, E], F16, tag="wo", name="wop")
            xtp0 = xp.tile([P, ESUB, QB], F16, tag="xt", name="xtp")
            # wq/x e-slices interleaved so Q(cc=0) can start ~2us in; the
            # rest of the first-chunk input streams underneath the matmuls
            for e in range(ESUB):
                nc.sync.dma_start(wqp[:, e, :], wq_r[:, e, :])
                nc.sync.dma_start(xtp0[:, e, :], xt_r[:, e, 0:QB])
            nc.sync.dma_start(bq_sb[:], bq_d[:])
            nc.sync.dma_start(bk_sb[:], bk_d[:])
            nc.sync.dma_start(bv_sb[:], bv_d[:])
            nc.sync.dma_start(tri_sb[:], tri_d[:])
            nc.sync.dma_start(on65_sb[:], on65_d[:])
            nc.sync.dma_start(wkp[:, 0:4, :], wk_r[:, 0:4, :])
            nc.sync.dma_start(wkp[:, 4:8, :], wk_r[:, 4:8, :])
            nc.sync.dma_start(wvp[:, 0:4, :], wv_r[:, 0:4, :])
            nc.sync.dma_start(wvp[:, 4:8, :], wv_r[:, 4:8, :])
            # ones column of every AV stationary, written once
            nc.vector.memset(
                vx_sb[:, :, :, D : D + 1].rearrange("p k h u -> p (k h) u"), 1.0
            )
            nc.sync.dma_start(wop[:], wo_d.rearrange("(o p) n -> p o n", p=P))

            def qk_part(qb, xtp):
                # e-outer/cc-inner: 4 matmuls per e-slice matches the DMA
                # arrival cadence at startup, so Q never stalls on wq slices
                # and K can start on the first wk half-load
                for w_p, b_sb, dst_sb, scale in (
                    (wqp, bq_sb, qt_sb, 0.125),
                    (wkp, bk_sb, kt_sb, 1.0),
                ):
                    t_a = bp.tile([P, 2, QB], F32, tag="work", name="qk_ps")
                    t_b = bp.tile([P, 2, QB], F32, tag="work", name="qk_ps")
                    for e in range(ESUB):
                        for cc in range(4):
                            nc.tensor.matmul(
                                (t_a if cc < 2 else t_b)[:, cc % 2, :],
                                lhsT=w_p[:, e, cc * P : (cc + 1) * P],
                                rhs=xtp[:, e, :],
                                start=(e == 0),
                                stop=(e == ESUB - 1),
                            )
                    for cc in range(4):
                        nc.scalar.activation(
                            dst_sb[:, cc, qb * QB : (qb + 1) * QB],
                            (t_a if cc < 2 else t_b)[:, cc % 2, :],
                            mybir.ActivationFunctionType.Identity,
                            bias=b_sb[:, cc : cc + 1],
                            scale=scale,
                        )

            def v_part(qb, xtp):
                for half in range(2):
                    v2 = bp.tile([P, 2, QB], F32, tag="work", name="v_ps")
                    for sub in range(2):
                        stl = half * 2 + sub
                        for e in range(ESUB):
                            nc.tensor.matmul(
                                v2[:, sub, :],
                                lhsT=xtp[:, e, stl * P : (stl + 1) * P],
                                rhs=wvp[:, e, :],
                                start=(e == 0),
                                stop=(e == ESUB - 1),
                            )
                    for sub in range(2):
                        st = qb * 4 + half * 2 + sub
                        nc.vector.tensor_tensor(
                            vx_sb[:, st, :, 0:D],
                            v2[:, sub, :].rearrange("p (h d) -> p h d", d=D),
                            bv_sb.rearrange("p (h d) -> p h d", d=D),
                            ADD,
                        )

            def attn_stage(qb, pairs=(0, 1, 2, 3), post_pair0=None):
                for pair in pairs:
                    if pair == 1 and post_pair0 is not None:
                        post_pair0()
                    heads = (2 * pair, 2 * pair + 1)
                    a_t = {
                        h: apl.tile([VW, QB], F32, tag="a", name="a_ps") for h in heads
                    }
                    nkt = 4 * qb + 4

                    def mk_av(a_t, heads, pair, qb, nkt):
                        def av_mms(pt_prev, kt_prev):
                            for h in heads:
                                nc.tensor.matmul(
                                    a_t[h][:],
                                    lhsT=vx_sb[:, kt_prev, h, :],
                                    rhs=pt_prev[:, h % 2, :],
                                    start=(kt_prev == 0),
                                    stop=(kt_prev == nkt - 1),
                                )
                            if kt_prev == nkt - 1:
                                normalize(a_t, heads, pair, qb)
                        return av_mms

                    av_mms = mk_av(a_t, heads, pair, qb, nkt)
                    for kt in range(nkt):
                        s_t = bp.tile([P, 2, QB], F32, tag="work", name="s_ps")
                        for h in heads:
                            hb = (h % 2) * D
                            nc.tensor.matmul(
                                s_t[:, h % 2, :],
                                lhsT=kt_sb[hb : hb + D, pair, kt * P : (kt + 1) * P],
                                rhs=qt_sb[hb : hb + D, pair, qb * QB : (qb + 1) * QB],
                                start=True,
                                stop=True,
                            )
                        pt = sp.tile([P, 2, QB], F16, tag="pt", name="pt", bufs=8)
                        m = kt - 4 * qb  # >= 0 only on diagonal k-tiles
                        if m > 0:
                            # dead left cols: zero on gpsimd BEFORE the exp
                            # writes the live region — off the exp->AV chain
                            nc.gpsimd.memset(pt[:, :, 0 : P * m], 0.0)
                            nc.scalar.activation(
                                pt[:, :, P * m :], s_t[:, :, P * m :], Exp
                            )
                        else:
                            nc.scalar.activation(pt[:], s_t[:], Exp)
                        if m >= 0:
                            # multiply the 128-wide triangle into both heads
                            # with a single DVE op
                            nc.vector.tensor_tensor(
                                pt[:, :, P * m : P * (m + 1)],
                                pt[:, :, P * m : P * (m + 1)],
                                tri_sb[:],
                                MULT,
                            )
                        if len(pending) >= AV_LAG:
                            pending.pop(0)()
                        pending.append(lambda pt=pt, kt=kt, f=av_mms: f(pt, kt))

            def normalize(a_t, heads, pair, qb):
                # one full-tile copy frees the PSUM bank in ~700ns; the
                # denominator DRAM-broadcast + reciprocal + scale run off
                # the SBUF copy, clear of the a_t bank recycle path
                fast = pair == 3 and qb == NQB - 1
                # odd head first: its chain ends in an at_sb DMA that gates
                # the output projection, so give it the head start
                order = (heads[1], heads[0])
                an = {}
                for h in order:
                    an[h] = sp.tile([VW, QB], F32, tag="dn", name="an", bufs=4)
                    nc.vector.tensor_copy(an[h][:], a_t[h][:])
                    if not fast:
                        nc.sync.dma_start(rc_d[h, qb : qb + 1, :], an[h][D:VW, :])
                for h in order:
                    hb = (h % 2) * D
                    rb = sp.tile([D, QB], F32, tag="rb", name="rb", bufs=2)
                    if fast:
                        # exact fp32 contract-1 matmul broadcasts the
                        # denominator row into the dead a_t bank: the kernel
                        # tail skips the DRAM round-trip entirely
                        nc.tensor.matmul(
                            a_t[h][0:D, :],
                            lhsT=on65_sb[D:VW, :],
                            rhs=an[h][D:VW, :],
                            start=True,
                            stop=True,
                        )
                        nc.vector.reciprocal_approx_fast(rb[:], a_t[h][0:D, :])
                    else:
                        db = sp.tile([D, QB], F32, tag="db", name="db", bufs=2)
                        nc.sync.dma_start(
                            db[:], rc_d[h, qb : qb + 1, :].to_broadcast([D, QB])
                        )
                        nc.vector.reciprocal_approx_fast(rb[:], db[:])
                    at_slice = at_sb[hb : hb + D, pair, qb * QB : (qb + 1) * QB]
                    if hb == 0:
                        nc.vector.tensor_tensor(at_slice, an[h][0:D, :], rb[:], MULT)
                    else:
                        tmp = sp.tile([D, QB], F16, tag="tmp", name="tmp", bufs=2)
                        nc.vector.tensor_tensor(tmp[:], an[h][0:D, :], rb[:], MULT)
                        nc.sync.dma_start(at_slice, tmp[:])

            def oproj_stage(qb):
                last = qb == NQB - 1
                for stl in range(4):
                    st = qb * 4 + stl
                    o = bp.tile([P, 2, QB], F32, tag="work", name="o_ps")
                    for n in range(2):
                        for t in range(4):
                            nc.tensor.matmul(
                                o[:, n, :],
                                lhsT=at_sb[:, t, st * P : (st + 1) * P],
                                rhs=wop[:, t, n * QB : (n + 1) * QB],
                                start=(t == 0),
                                stop=(t == 3),
                            )
                    o_sb = sp.tile(
                        [P, 2, QB], F16 if last else F32, tag="ob",
                        name="o_sb", bufs=2,
                    )
                    for n in range(2):
                        if (st + n) % 2 == 0:
                            nc.scalar.copy(o_sb[:, n, :], o[:, n, :])
                        else:
                            nc.vector.tensor_copy(o_sb[:, n, :], o[:, n, :])
                        dst = (
                            out2_d[stl * P : (stl + 1) * P, n * QB : (n + 1) * QB]
                            if last
                            else out_d[st * P : (st + 1) * P, n * QB : (n + 1) * QB]
                        )
                        nc.sync.dma_start(dst, o_sb[:, n, :])

            pending = []
            for qb in range(NQB):
                if qb + 1 < NQB:
                    nxt = xp.tile([P, ESUB, QB], F16, tag="xt", name="xtp")
                    nc.sync.dma_start(
                        nxt[:], xt_r[:, :, (qb + 1) * QB : (qb + 2) * QB]
                    )
                else:
                    nxt = None
                xtp = xtp0 if qb == 0 else cur
                qk_part(qb, xtp)
                v_part(qb, xtp)
                attn_stage(
                    qb,
                    post_pair0=(
                        (lambda q=qb: oproj_stage(q - 1)) if qb >= 1 else None
                    ),
                )
                cur = nxt
                if qb == NQB - 1:
                    for f in pending:
                        f()
                    pending = []
            oproj_stage(NQB - 1)

    nc.compile()
    return nc


def _prep_inputs(x, Wqkv, bqkv, Wo, bo):
    bf16 = np.float16
    x = np.asarray(x, np.float32)
    Wqkv = np.asarray(Wqkv, np.float32)
    bqkv = np.asarray(bqkv, np.float32)
    Wo = np.asarray(Wo, np.float32)

    kl = np.arange(P)[:, None]
    tri1 = (kl <= np.arange(P)[None, :]).astype(np.float16)
    tri = np.stack([tri1, tri1], axis=1)
    in_maps = []
    for c in range(8):
        b, g = divmod(c, 2)
        lo, hi = g * G, (g + 1) * G
        in_maps.append(
            {
                "xt": np.ascontiguousarray(x[b].T).astype(bf16),
                "wq": np.ascontiguousarray(Wqkv[:, lo:hi]).astype(bf16),
                "wk": np.ascontiguousarray(Wqkv[:, E + lo : E + hi]).astype(bf16),
                "wv": np.ascontiguousarray(Wqkv[:, 2 * E + lo : 2 * E + hi]).astype(
                    bf16
                ),
                "wo": np.ascontiguousarray(Wo[lo:hi, :]).astype(bf16),
                "bq": np.ascontiguousarray(bqkv[lo:hi].reshape(4, P).T) * 0.125,
                "bk": np.ascontiguousarray(bqkv[E + lo : E + hi].reshape(4, P).T),
                "bv": np.tile(bqkv[2 * E + lo : 2 * E + hi][None, :], (P, 1)).astype(
                    np.float32
                ),
                "tri": tri,
                "on65": np.ones((VW, D), np.float32),
            }
        )
    return in_maps


def kernel(x, Wqkv, bqkv, Wo, bo, _trace=False):
    if "nc" not in _CACHE:
        _CACHE["nc"] = _build_program()
    nc = _CACHE["nc"]

    in_maps = _prep_inputs(x, Wqkv, bqkv, Wo, bo)
    res = run_bass_kernel_spmd(nc, in_maps, core_ids=list(range(8)), trace=_trace)
    _CACHE["last_result"] = res

    bo = np.asarray(bo, np.float32)
    out = np.empty((B, S, E), np.float32)
    for b in range(B):
        out[b] = (
            res.results[2 * b]["out"].astype(np.float32)
            + res.results[2 * b + 1]["out"].astype(np.float32)
            + bo
        )
        out[b, S - QB :] = (
            res.results[2 * b]["out2"].astype(np.float32)
            + res.results[2 * b + 1]["out2"].astype(np.float32)
            + bo
        )
    return out
